# revision 44
# baseline (speedup 1.0000x reference)
"""Trainium2 Bass kernel for an 8-layer LoRA attention model.

Model (per layer): Wq_m = Wqkv + Aqkv@Bqkv; qkv = h @ Wq_m; causal MHA
(16 heads, hd=64); Wp_m = Wproj + Aproj@Bproj; h = h + attn_out @ Wp_m.
x: [2, 1024, 1024] fp32.

Distribution (8 cores): DP2 x TP4. Cores 0-3 handle batch 0, cores 4-7
batch 1. Within a group each core owns 4 heads: a 256-column shard of
Wq/Wk/Wv (Megatron split) and a 256-OUTPUT-column shard of Wproj. Per
layer the group does two AllGathers (attention outputs, then proj
outputs); there is no reduce, so results are bit-identical across the
group and numerically equal to the unsharded computation.

Layout: the residual stream is kept transposed on-chip: hT [1024 d, 1024
tokens]. The qk GEMM produces qT/kT directly ([dims, tokens]); v is
produced untransposed ([tokens, dims]) so it can serve as the stationary
operand of the attention AV matmul. Attention uses the scoresT [s2, s1]
orientation; softmax row-sums come from a ones-column appended to v
(AV output row 64 = sum of exp), so no partition-dim reductions or
transposes are ever needed.

Activations run in float32r (fp32 storage, ~12-bit mantissa operand
rounding, full PE rate); weights are shipped and used as bf16 stationary
operands (same PE rate, half the wire/DMA/LDWEIGHTS cost). All matmuls
accumulate in fp32 PSUM.
"""

import numpy as np

import concourse.bass as bass
import concourse.mybir as mybir
import concourse.tile as tile
from concourse import bacc
from concourse.bass_utils import run_bass_kernel_spmd

F32 = mybir.dt.float32
BF16 = mybir.dt.bfloat16
F32R = mybir.dt.float32r
F8 = mybir.dt.float8e4
EXP = mybir.ActivationFunctionType.Exp

L = 8          # layers
D = 1024       # model dim
S = 1024       # sequence (tokens per batch == tokens per core)
HD = 64        # head dim
HPC = 4        # heads per core
HL = HPC * HD  # local head dims (256)
TP = 4         # tensor-parallel group size
N_CORES = 8
GROUPS = [[0, 1, 2, 3], [4, 5, 6, 7]]
NEG = -1.0e30

KT = D // 128   # k tiles of the model dim (8)
NJ = 2          # token chunks (512 each)
CW = S // NJ    # chunk width (512)


def build_program(n_layers: int = L, reps: int = 1, no_cc: bool = False,
                  num_devices: int = N_CORES, wire_bf16: bool = True,
                  attn_bf16: bool = True, gps_bufs: int = 2, avps_bufs: int = 2,
                  fast_recip: bool = True, mask_select: bool = True,
                  o_fp8: bool = False):
    nc = bacc.Bacc("TRN2", target_bir_lowering=False, debug=False,
                   num_devices=num_devices)
    WIRE = BF16 if wire_bf16 else F32R
    WIRE_O = F8 if o_fp8 else WIRE  # attention-output wire (AG + proj moving)
    ABF = BF16 if attn_bf16 else F32R

    # ---- per-core external inputs (host pre-sharded; weights bf16) ----
    xT = nc.dram_tensor("xT", [D, S], F32R, kind="ExternalInput")
    wqk_d = nc.dram_tensor("wqk", [n_layers, D, 2 * HL], BF16, kind="ExternalInput")
    wv_d = nc.dram_tensor("wv", [n_layers, D, HL], BF16, kind="ExternalInput")
    wp_d = nc.dram_tensor("wp", [n_layers, D, HL], BF16, kind="ExternalInput")
    aqt_d = nc.dram_tensor("aqt", [n_layers, 16, D], BF16, kind="ExternalInput")
    bqk_d = nc.dram_tensor("bqk", [n_layers, 16, 2 * HL], BF16, kind="ExternalInput")
    bv_d = nc.dram_tensor("bv", [n_layers, 16, HL], BF16, kind="ExternalInput")
    apt_d = nc.dram_tensor("apt", [n_layers, 16, D], BF16, kind="ExternalInput")
    bp_d = nc.dram_tensor("bp", [n_layers, 16, HL], BF16, kind="ExternalInput")
    outT = nc.dram_tensor("outT", [D, S], F32R, kind="ExternalOutput")

    with tile.TileContext(nc) as tc:
        from contextlib import ExitStack
        with ExitStack() as _st:
            _p = lambda *a, **kw: _st.enter_context(tc.tile_pool(*a, **kw))
            per = _p(name="per", bufs=1)          # persistent
            wpool = _p(name="wpool", bufs=2)      # qk weights (prefetch)
            wpool1 = _p(name="wpool1", bufs=2)    # v/p weights (prefetch)
            lora = _p(name="lora", bufs=2)
            lora2 = _p(name="lora2", bufs=2)
            expp = _p(name="expp", bufs=3)
            opool = _p(name="opool", bufs=2)      # o chunks + norm temps
            gat = _p(name="gat", bufs=1)          # gathered o (per chunk)
            pf = _p(name="pf", bufs=2 if wire_bf16 else 1)  # gathered proj
            prs = _p(name="prs", bufs=2)          # proj shard staging
            qkv2 = _p(name="qkv2", bufs=2)
            gps = _p(name="gps", bufs=gps_bufs, space="PSUM")
            projps = _p(name="projps", bufs=2, space="PSUM")
            sps = _p(name="sps", bufs=2, space="PSUM")
            avps = _p(name="avps", bufs=avps_bufs, space="PSUM")
            dram = _p(name="dram", bufs=2, space="DRAM")
            # ---- persistent state ----
            hT = [per.tile([128, S], F32R, tag=f"hT{k}", name=f"hT{k}") for k in range(KT)]
            # bf16 shadow of the residual: the matmul-operand copy
            hbf = [per.tile([128, S], BF16, tag=f"hbf{k}", name=f"hbf{k}")
                   for k in range(KT)]

            ones_sb = per.tile([128, HPC, 1], ABF, tag="ones", name="ones_sb")
            nc.gpsimd.memset(ones_sb[:], 1.0)

            # causal bias: [128,128] lower-strict triangle = NEG, else 0
            maskb = per.tile([128, 128], F32, tag="maskb", name="maskb")
            nc.gpsimd.memset(maskb[:], 0.0)
            nc.gpsimd.affine_select(
                out=maskb[:], in_=maskb[:],
                compare_op=mybir.AluOpType.is_ge,
                fill=NEG, base=0,
                pattern=[[1, 128]], channel_multiplier=-1,
            )

            for rep in range(reps):
              for jk in range(NJ * KT):
                  j, k = divmod(jk, KT)
                  jsl = slice(j * CW, (j + 1) * CW)
                  nc.sync.dma_start(hT[k][:, jsl],
                                    xT[k * 128:(k + 1) * 128, jsl])
                  nc.vector.tensor_copy(hbf[k][:, jsl], hT[k][:, jsl])
              for l in range(n_layers):
                # qkT [512 rows, S]: rows 0..255 = qT (4 heads x 64),
                # 256..511 = kT.  4 tiles of [128, S].  Double-buffered so
                # next layer's GEMMs overlap this layer's attention reads.
                qkT = [qkv2.tile([128, S], ABF, tag=f"qkT{m}", name=f"qkT{m}")
                       for m in range(4)]
                # v_loc per s2-tile: [128, 4*65]; per head 64 v-dims + ones col
                v_loc = [qkv2.tile([128, HPC * (HD + 1)], ABF, tag=f"v{i}",
                                   name=f"v{i}") for i in range(S // 128)]
                for i in range(S // 128):
                    vv = v_loc[i][:].rearrange("p (h e) -> p h e", e=HD + 1)
                    nc.vector.tensor_copy(vv[:, :, HD:HD + 1], ones_sb[:])
                # ================= weights + LoRA merge =================
                wqk_all = wpool.tile([128, KT, 2 * HL], BF16, tag="wqk",
                                     name="wqk_all")
                wv_all = wpool1.tile([128, KT, HL], BF16, tag="wv", name="wv_all")
                wp_all = wpool1.tile([128, KT, HL], BF16, tag="wp", name="wp_all")
                nc.sync.dma_start(
                    wqk_all[:], wqk_d[l].rearrange("(k p) n -> p k n", p=128))
                nc.sync.dma_start(
                    wv_all[:], wv_d[l].rearrange("(k p) n -> p k n", p=128))
                nc.sync.dma_start(
                    wp_all[:], wp_d[l].rearrange("(k p) n -> p k n", p=128))
                wqk = [wqk_all[:, k, :] for k in range(KT)]
                wv = [wv_all[:, k, :] for k in range(KT)]
                wp = [wp_all[:, k, :] for k in range(KT)]
                aqt = lora.tile([16, D], BF16, tag="aqt", name="aqt")
                apt = lora2.tile([16, D], BF16, tag="apt", name="apt")
                nc.sync.dma_start(aqt[:], aqt_d[l])
                nc.sync.dma_start(apt[:], apt_d[l])
                bqk = lora.tile([16, 2 * HL], BF16, tag="bqk", name="bqk")
                bv = lora.tile([16, HL], BF16, tag="bv", name="bv")
                bp = lora2.tile([16, HL], BF16, tag="bp", name="bp")
                nc.sync.dma_start(bqk[:], bqk_d[l])
                nc.sync.dma_start(bv[:], bv_d[l])
                nc.sync.dma_start(bp[:], bp_d[l])

                # LoRA merge (runs a layer ahead, off the critical path):
                # W[k] += A^T[:, k].T @ B for each 128-row tile of each W
                for k in range(KT):
                    ksl = slice(k * 128, (k + 1) * 128)
                    mq = gps.tile([128, 2 * HL], F32, tag="mm", name="mq")
                    nc.tensor.matmul(mq[:], aqt[:, ksl], bqk[:],
                                     start=True, stop=True)
                    nc.vector.tensor_add(wqk[k], wqk[k], mq[:])
                    mv = gps.tile([128, HL], F32, tag="mm", name="mv")
                    nc.tensor.matmul(mv[:], aqt[:, ksl], bv[:],
                                     start=True, stop=True)
                    nc.vector.tensor_add(wv[k], wv[k], mv[:])
                    mp = gps.tile([128, HL], F32, tag="mm", name="mp")
                    nc.tensor.matmul(mp[:], apt[:, ksl], bp[:],
                                     start=True, stop=True)
                    nc.vector.tensor_add(wp[k], wp[k], mp[:])

                def _attn_chunk(j, jeng):
                    n_i = 4 * j + 4  # causal: s2-tiles 0 .. 4j+3
                    for pair in range(2):
                        av = [avps.tile([HD + 1, CW], F32, tag="av", name="av")
                              for _ in range(2)]
                        for i in range(n_i):
                            sq = i - 4 * j  # >=0 on the diagonal band
                            c0 = max(sq, 0) * 128
                            for h2 in range(2):
                                hsl = slice(64 * h2, 64 * h2 + 64)
                                ps_s = sps.tile([128, CW], F32, tag="sc")
                                nc.tensor.matmul(
                                    ps_s[:, c0:CW],
                                    qkT[2 + pair][hsl, i * 128:(i + 1) * 128],
                                    qkT[pair][hsl, j * CW + c0:(j + 1) * CW],
                                    start=True, stop=True,
                                    tile_position=(64 * h2, 0),
                                )
                                if sq >= 0 and not mask_select:
                                    nc.vector.tensor_add(
                                        ps_s[:, c0:c0 + 128],
                                        ps_s[:, c0:c0 + 128],
                                        maskb[:],
                                    )
                                e = expp.tile([128, CW], ABF, tag="e")
                                nc.scalar.activation(out=e[:, c0:CW],
                                                     in_=ps_s[:, c0:CW],
                                                     func=EXP, scale=0.125)
                                if sq >= 0 and mask_select:
                                    # causal: zero exp() on the strict lower
                                    # triangle of the diagonal block
                                    nc.gpsimd.affine_select(
                                        out=e[:, c0:c0 + 128],
                                        in_=e[:, c0:c0 + 128],
                                        compare_op=mybir.AluOpType.is_ge,
                                        fill=0.0, base=0,
                                        pattern=[[1, 128]], channel_multiplier=-1,
                                    )
                                h = 2 * pair + h2
                                nc.tensor.matmul(
                                    av[h2][:, c0:CW],
                                    v_loc[i][:, h * (HD + 1):(h + 1) * (HD + 1)],
                                    e[:, c0:CW],
                                    start=(i == 0), stop=(i == n_i - 1),
                                )
                        # stage av to SBUF (frees the PSUM tile for the next
                        # pair), then normalize: o = av[0:HD] / av[HD].
                        # zrow must sit at partition 0: the approx-recip
                        # custom DVE op misbehaves on offset-64 sources.
                        for h2 in range(2):
                            h = 2 * pair + h2
                            avs = opool.tile([HD, CW], F32, tag="avs")
                            nc.scalar.copy(avs[:], av[h2][0:HD, :])
                            zrow = opool.tile([1, CW], F32, tag="zrow")
                            nc.scalar.copy(zrow[:], av[h2][HD:HD + 1, :])
                            recip = opool.tile([1, CW], F32, tag="recip")
                            if fast_recip:
                                nc.vector.reciprocal_approx_fast(
                                    out=recip[:], in_=zrow[:])
                            else:
                                nc.vector.reciprocal(recip[:], zrow[:])
                            rbc = opool.tile([HD, CW], F32, tag="rbc")
                            nc.gpsimd.partition_broadcast(rbc[:], recip[:])
                            o_j = opool.tile([HD, CW], WIRE_O, tag="o_j")
                            nc.vector.tensor_mul(o_j[:], avs[0:HD, :], rbc[:])
                            jeng.dma_start(o_shard[j][h * HD:(h + 1) * HD, :],
                                           o_j[:])

                # ====== chunk-major: qkT/v/attention/AG_o per chunk ======
                o_shard = [dram.tile([HL, CW], WIRE_O, tag=f"o_shard{j}",
                                     name=f"o_shard{j}") for j in range(NJ)]
                o_full_d = []
                for j in range(NJ):
                    jsl = slice(j * CW, (j + 1) * CW)
                    jeng = nc.sync if j == 0 else nc.scalar
                    # qkT[m rows, j] = wqk_m[:, m].T @ h[:, j]
                    for m in range(4):
                        msl = slice(m * 128, (m + 1) * 128)
                        ps = gps.tile([128, CW], F32, tag="mm")
                        for k in range(KT):
                            nc.tensor.matmul(ps[:], wqk[k][:, msl], hbf[k][:, jsl],
                                             start=(k == 0), stop=(k == KT - 1))
                        nc.scalar.copy(qkT[m][:, jsl], ps[:])

                    # v[i, head dims] = h[:, i].T @ wv_m
                    for i in range(4 * j, 4 * j + 4):
                        isl = slice(i * 128, (i + 1) * 128)
                        ps = gps.tile([128, HL], F32, tag="mm")
                        for k in range(KT):
                            nc.tensor.matmul(ps[:], hbf[k][:, isl], wv[k],
                                             start=(k == 0), stop=(k == KT - 1))
                        vv = v_loc[i][:].rearrange("p (h e) -> p h e", e=HD + 1)
                        nc.vector.tensor_copy(
                            vv[:, :, 0:HD],
                            ps[:].rearrange("p (h e) -> p h e", e=HD))

                    # attention chunk j
                    _attn_chunk(j, jeng)

                    # AG_o(j) fires as soon as chunk j's heads are written
                    ofd = dram.tile([D, CW], WIRE_O, tag=f"o_full{j}",
                                    name=f"o_full{j}")
                    o_full_d.append(ofd)
                    if no_cc == 2:
                        jeng.dma_start(ofd[0:HL, :], o_shard[j][:])
                    elif no_cc:
                        for q in range(TP):
                            jeng.dma_start(ofd[q * HL:(q + 1) * HL, :],
                                           o_shard[j][:])
                    else:
                        nc.gpsimd.collective_compute(
                            "AllGather", mybir.AluOpType.bypass,
                            replica_groups=GROUPS,
                            ins=[o_shard[j].opt()], outs=[ofd.opt()],
                        )

                # ====== proj/AG p per chunk -> residual ======
                p_full_d = []
                for j in range(NJ):
                    jeng = nc.sync if j == 0 else nc.scalar
                    ofull_all = gat.tile([128, KT, CW], WIRE_O, tag="of",
                                         name="ofull_all")
                    jeng.dma_start(
                        ofull_all[:],
                        o_full_d[j][:].rearrange("(k p) n -> p k n", p=128))
                    ofull = [ofull_all[:, k, :] for k in range(KT)]
                    p_shard = dram.tile([HL, CW], WIRE, tag=f"p_shard{j}",
                                        name=f"p_shard{j}")
                    for m in range(2):
                        msl = slice(m * 128, (m + 1) * 128)
                        ps = projps.tile([128, CW], F32, tag="pmm")
                        for k in range(KT):
                            nc.tensor.matmul(ps[:], wp[k][:, msl], ofull[k],
                                             start=(k == 0), stop=(k == KT - 1))
                        prm = prs.tile([128, CW], WIRE, tag="pr", name="pr")
                        nc.scalar.copy(prm[:], ps[:])
                        jeng.dma_start(p_shard[m * 128:(m + 1) * 128, :],
                                       prm[:])

                    pfd = dram.tile([D, CW], WIRE, tag=f"p_full{j}",
                                    name=f"p_full{j}")
                    p_full_d.append(pfd)
                    if no_cc == 2:
                        jeng.dma_start(pfd[0:HL, :], p_shard[:])
                    elif no_cc:
                        for q in range(TP):
                            jeng.dma_start(pfd[q * HL:(q + 1) * HL, :],
                                           p_shard[:])
                    else:
                        nc.gpsimd.collective_compute(
                            "AllGather", mybir.AluOpType.bypass,
                            replica_groups=GROUPS,
                            ins=[p_shard.opt()], outs=[pfd.opt()],
                        )
                for j in range(NJ):
                    jsl = slice(j * CW, (j + 1) * CW)
                    jeng = nc.sync if j == 0 else nc.scalar
                    pf_all = pf.tile([128, KT, CW], WIRE, tag="pf", name="pf_all")
                    jeng.dma_start(
                        pf_all[:],
                        p_full_d[j][:].rearrange("(k p) n -> p k n", p=128))
                    for k in range(KT):
                        nc.vector.tensor_add(hT[k][:, jsl], hT[k][:, jsl],
                                             pf_all[:, k, :])
                        if l == n_layers - 1:
                            jeng.dma_start(outT[k * 128:(k + 1) * 128, jsl],
                                           hT[k][:, jsl])
                        else:
                            nc.vector.tensor_copy(hbf[k][:, jsl],
                                                  hT[k][:, jsl])

    nc.compile()
    return nc


def make_in_maps(inputs: dict, n_layers: int = L):
    import ml_dtypes
    BF = ml_dtypes.bfloat16
    x = np.asarray(inputs["x"], np.float32)
    Wqkv = np.asarray(inputs["Wqkv"]).astype(BF)[:n_layers]
    Aqkv = np.asarray(inputs["Aqkv"]).astype(BF)[:n_layers]
    Bqkv = np.asarray(inputs["Bqkv"]).astype(BF)[:n_layers]
    Wproj = np.asarray(inputs["Wproj"]).astype(BF)[:n_layers]
    Aproj = np.asarray(inputs["Aproj"]).astype(BF)[:n_layers]
    Bproj = np.asarray(inputs["Bproj"]).astype(BF)[:n_layers]

    in_maps = []
    for c in range(N_CORES):
        b, t = c // TP, c % TP
        cs = slice(HL * t, HL * t + HL)  # this core's head-dim columns
        wqk = np.concatenate([Wqkv[:, :, cs], Wqkv[:, :, D + HL * t:D + HL * t + HL]],
                             axis=2)
        bqk = np.concatenate([Bqkv[:, :, cs], Bqkv[:, :, D + HL * t:D + HL * t + HL]],
                             axis=2)
        in_maps.append({
            "xT": np.ascontiguousarray(x[b].T),
            "wqk": np.ascontiguousarray(wqk),
            "wv": np.ascontiguousarray(Wqkv[:, :, 2 * D + HL * t:2 * D + HL * t + HL]),
            "wp": np.ascontiguousarray(Wproj[:, :, cs]),
            "aqt": np.ascontiguousarray(Aqkv.transpose(0, 2, 1)),
            "bqk": bqk,
            "bv": np.ascontiguousarray(Bqkv[:, :, 2 * D + HL * t:2 * D + HL * t + HL]),
            "apt": np.ascontiguousarray(Aproj.transpose(0, 2, 1)),
            "bp": np.ascontiguousarray(Bproj[:, :, cs]),
        })
    return in_maps


_NC_CACHE = {}


def kernel(**inputs) -> np.ndarray:
    n_layers = L
    if n_layers not in _NC_CACHE:
        _NC_CACHE[n_layers] = build_program(n_layers)
    nc = _NC_CACHE[n_layers]
    in_maps = make_in_maps(inputs, n_layers)
    res = run_bass_kernel_spmd(nc, in_maps, core_ids=list(range(N_CORES)))
    out0 = res.results[0]["outT"].T
    out1 = res.results[TP]["outT"].T
    return np.stack([out0, out1]).astype(np.float32)


if __name__ == "__main__":
    rng = np.random.default_rng(0)
    s = 0.02
    inputs = {
        "x": rng.standard_normal((2, S, D)).astype(np.float32),
        "Wqkv": (rng.standard_normal((L, D, 3 * D)) * s).astype(np.float32),
        "Aqkv": (rng.standard_normal((L, D, 16)) * s).astype(np.float32),
        "Bqkv": (rng.standard_normal((L, 16, 3 * D)) * s).astype(np.float32),
        "Wproj": (rng.standard_normal((L, D, D)) * s).astype(np.float32),
        "Aproj": (rng.standard_normal((L, D, 16)) * s).astype(np.float32),
        "Bproj": (rng.standard_normal((L, 16, D)) * s).astype(np.float32),
    }
    out = kernel(**inputs)
    print("kernel output:", out.shape, out.dtype, float(np.abs(out).max()))



# revision 48
# speedup vs baseline: 1.0066x; 1.0066x over previous
"""Trainium2 Bass kernel for an 8-layer LoRA attention model.

Model (per layer): Wq_m = Wqkv + Aqkv@Bqkv; qkv = h @ Wq_m; causal MHA
(16 heads, hd=64); Wp_m = Wproj + Aproj@Bproj; h = h + attn_out @ Wp_m.
x: [2, 1024, 1024] fp32.

Distribution (8 cores): DP2 x TP4. Cores 0-3 handle batch 0, cores 4-7
batch 1. Within a group each core owns 4 heads: a 256-column shard of
Wq/Wk/Wv (Megatron split) and a 256-OUTPUT-column shard of Wproj. Per
layer the group does two AllGathers (attention outputs, then proj
outputs); there is no reduce, so results are bit-identical across the
group and numerically equal to the unsharded computation.

Layout: the residual stream is kept transposed on-chip: hT [1024 d, 1024
tokens]. The qk GEMM produces qT/kT directly ([dims, tokens]); v is
produced untransposed ([tokens, dims]) so it can serve as the stationary
operand of the attention AV matmul. Attention uses the scoresT [s2, s1]
orientation; softmax row-sums come from a ones-column appended to v
(AV output row 64 = sum of exp), so no partition-dim reductions or
transposes are ever needed.

Activations run in float32r (fp32 storage, ~12-bit mantissa operand
rounding, full PE rate); weights are shipped and used as bf16 stationary
operands (same PE rate, half the wire/DMA/LDWEIGHTS cost). All matmuls
accumulate in fp32 PSUM.
"""

import numpy as np

import concourse.bass as bass
import concourse.mybir as mybir
import concourse.tile as tile
from concourse import bacc
from concourse.bass_utils import run_bass_kernel_spmd

F32 = mybir.dt.float32
BF16 = mybir.dt.bfloat16
F32R = mybir.dt.float32r
F8 = mybir.dt.float8e4
EXP = mybir.ActivationFunctionType.Exp

L = 8          # layers
D = 1024       # model dim
S = 1024       # sequence (tokens per batch == tokens per core)
HD = 64        # head dim
HPC = 4        # heads per core
HL = HPC * HD  # local head dims (256)
TP = 4         # tensor-parallel group size
N_CORES = 8
GROUPS = [[0, 1, 2, 3], [4, 5, 6, 7]]
NEG = -1.0e30

KT = D // 128   # k tiles of the model dim (8)
NJ = 2          # token chunks (512 each)
CW = S // NJ    # chunk width (512)


def build_program(n_layers: int = L, reps: int = 1, no_cc: bool = False,
                  num_devices: int = N_CORES, wire_bf16: bool = True,
                  attn_bf16: bool = True, gps_bufs: int = 2, avps_bufs: int = 2,
                  fast_recip: bool = True, mask_select: bool = True,
                  o_fp8: bool = False):
    nc = bacc.Bacc("TRN2", target_bir_lowering=False, debug=False,
                   num_devices=num_devices)
    WIRE = BF16 if wire_bf16 else F32R
    WIRE_O = F8 if o_fp8 else WIRE  # attention-output wire (AG + proj moving)
    ABF = BF16 if attn_bf16 else F32R

    # ---- per-core external inputs (host pre-sharded; weights bf16) ----
    xT = nc.dram_tensor("xT", [D, S], F32R, kind="ExternalInput")
    wqk_d = nc.dram_tensor("wqk", [n_layers, D, 2 * HL], BF16, kind="ExternalInput")
    wv_d = nc.dram_tensor("wv", [n_layers, D, HL], BF16, kind="ExternalInput")
    wp_d = nc.dram_tensor("wp", [n_layers, D, HL], BF16, kind="ExternalInput")
    aqt_d = nc.dram_tensor("aqt", [n_layers, 16, D], BF16, kind="ExternalInput")
    bqk_d = nc.dram_tensor("bqk", [n_layers, 16, 2 * HL], BF16, kind="ExternalInput")
    bv_d = nc.dram_tensor("bv", [n_layers, 16, HL], BF16, kind="ExternalInput")
    apt_d = nc.dram_tensor("apt", [n_layers, 16, D], BF16, kind="ExternalInput")
    bp_d = nc.dram_tensor("bp", [n_layers, 16, HL], BF16, kind="ExternalInput")
    outT = nc.dram_tensor("outT", [D, S], F32R, kind="ExternalOutput")

    with tile.TileContext(nc) as tc:
        from contextlib import ExitStack
        with ExitStack() as _st:
            _p = lambda *a, **kw: _st.enter_context(tc.tile_pool(*a, **kw))
            per = _p(name="per", bufs=1)          # persistent
            wpool = _p(name="wpool", bufs=2)      # qk weights (prefetch)
            wpool1 = _p(name="wpool1", bufs=2)    # v/p weights (prefetch)
            lora = _p(name="lora", bufs=2)
            lora2 = _p(name="lora2", bufs=2)
            expp = _p(name="expp", bufs=3)
            opool = _p(name="opool", bufs=2)      # o chunks + norm temps
            gat = _p(name="gat", bufs=1)          # gathered o (per chunk)
            pf = _p(name="pf", bufs=2 if wire_bf16 else 1)  # gathered proj
            prs = _p(name="prs", bufs=2)          # proj shard staging
            qkv2 = _p(name="qkv2", bufs=2)
            gps = _p(name="gps", bufs=gps_bufs, space="PSUM")
            projps = _p(name="projps", bufs=2, space="PSUM")
            sps = _p(name="sps", bufs=2, space="PSUM")
            avps = _p(name="avps", bufs=avps_bufs, space="PSUM")
            dram = _p(name="dram", bufs=2, space="DRAM")
            # ---- persistent state ----
            hT = [per.tile([128, S], F32R, tag=f"hT{k}", name=f"hT{k}") for k in range(KT)]
            # bf16 shadow of the residual: the matmul-operand copy
            hbf = [per.tile([128, S], BF16, tag=f"hbf{k}", name=f"hbf{k}")
                   for k in range(KT)]

            ones_sb = per.tile([128, HPC, 1], ABF, tag="ones", name="ones_sb")
            nc.gpsimd.memset(ones_sb[:], 1.0)

            # causal bias: [128,128] lower-strict triangle = NEG, else 0
            maskb = per.tile([128, 128], F32, tag="maskb", name="maskb")
            nc.gpsimd.memset(maskb[:], 0.0)
            nc.gpsimd.affine_select(
                out=maskb[:], in_=maskb[:],
                compare_op=mybir.AluOpType.is_ge,
                fill=NEG, base=0,
                pattern=[[1, 128]], channel_multiplier=-1,
            )

            for rep in range(reps):
              for jk in range(NJ * KT):
                  j, k = divmod(jk, KT)
                  jsl = slice(j * CW, (j + 1) * CW)
                  nc.sync.dma_start(hT[k][:, jsl],
                                    xT[k * 128:(k + 1) * 128, jsl])
                  nc.vector.tensor_copy(hbf[k][:, jsl], hT[k][:, jsl])
              for l in range(n_layers):
                # qkT [512 rows, S]: rows 0..255 = qT (4 heads x 64),
                # 256..511 = kT.  4 tiles of [128, S].  Double-buffered so
                # next layer's GEMMs overlap this layer's attention reads.
                qkT = [qkv2.tile([128, S], ABF, tag=f"qkT{m}", name=f"qkT{m}")
                       for m in range(4)]
                # v_loc per s2-tile: [128, 4*65]; per head 64 v-dims + ones col
                v_loc = [qkv2.tile([128, HPC * (HD + 1)], ABF, tag=f"v{i}",
                                   name=f"v{i}") for i in range(S // 128)]
                for i in range(S // 128):
                    vv = v_loc[i][:].rearrange("p (h e) -> p h e", e=HD + 1)
                    nc.vector.tensor_copy(vv[:, :, HD:HD + 1], ones_sb[:])
                # ================= weights + LoRA merge =================
                wqk_all = wpool.tile([128, KT, 2 * HL], BF16, tag="wqk",
                                     name="wqk_all")
                wv_all = wpool1.tile([128, KT, HL], BF16, tag="wv", name="wv_all")
                wp_all = wpool1.tile([128, KT, HL], BF16, tag="wp", name="wp_all")
                nc.sync.dma_start(
                    wqk_all[:], wqk_d[l].rearrange("(k p) n -> p k n", p=128))
                nc.sync.dma_start(
                    wv_all[:], wv_d[l].rearrange("(k p) n -> p k n", p=128))
                nc.sync.dma_start(
                    wp_all[:], wp_d[l].rearrange("(k p) n -> p k n", p=128))
                wqk = [wqk_all[:, k, :] for k in range(KT)]
                wv = [wv_all[:, k, :] for k in range(KT)]
                wp = [wp_all[:, k, :] for k in range(KT)]
                aqt = lora.tile([16, D], BF16, tag="aqt", name="aqt")
                apt = lora2.tile([16, D], BF16, tag="apt", name="apt")
                nc.sync.dma_start(aqt[:], aqt_d[l])
                nc.sync.dma_start(apt[:], apt_d[l])
                bqk = lora.tile([16, 2 * HL], BF16, tag="bqk", name="bqk")
                bv = lora.tile([16, HL], BF16, tag="bv", name="bv")
                bp = lora2.tile([16, HL], BF16, tag="bp", name="bp")
                nc.sync.dma_start(bqk[:], bqk_d[l])
                nc.sync.dma_start(bv[:], bv_d[l])
                nc.sync.dma_start(bp[:], bp_d[l])

                # LoRA merge (runs a layer ahead, off the critical path):
                # W[k] += A^T[:, k].T @ B for each 128-row tile of each W
                for k in range(KT):
                    ksl = slice(k * 128, (k + 1) * 128)
                    mq = gps.tile([128, 2 * HL], F32, tag="mm", name="mq")
                    nc.tensor.matmul(mq[:], aqt[:, ksl], bqk[:],
                                     start=True, stop=True)
                    nc.vector.tensor_add(wqk[k], wqk[k], mq[:])
                    mv = gps.tile([128, HL], F32, tag="mm", name="mv")
                    nc.tensor.matmul(mv[:], aqt[:, ksl], bv[:],
                                     start=True, stop=True)
                    nc.vector.tensor_add(wv[k], wv[k], mv[:])
                    mp = gps.tile([128, HL], F32, tag="mm", name="mp")
                    nc.tensor.matmul(mp[:], apt[:, ksl], bp[:],
                                     start=True, stop=True)
                    nc.vector.tensor_add(wp[k], wp[k], mp[:])

                def _attn_chunk(j, jeng):
                    n_i = 4 * j + 4  # causal: s2-tiles 0 .. 4j+3
                    for pair in range(2):
                        av = [avps.tile([HD + 1, CW], F32, tag="av", name="av")
                              for _ in range(2)]
                        for i in range(n_i):
                            sq = i - 4 * j  # >=0 on the diagonal band
                            c0 = max(sq, 0) * 128
                            for h2 in range(2):
                                hsl = slice(64 * h2, 64 * h2 + 64)
                                ps_s = sps.tile([128, CW], F32, tag="sc")
                                nc.tensor.matmul(
                                    ps_s[:, c0:CW],
                                    qkT[2 + pair][hsl, i * 128:(i + 1) * 128],
                                    qkT[pair][hsl, j * CW + c0:(j + 1) * CW],
                                    start=True, stop=True,
                                    tile_position=(64 * h2, 0),
                                )
                                if sq >= 0 and not mask_select:
                                    nc.vector.tensor_add(
                                        ps_s[:, c0:c0 + 128],
                                        ps_s[:, c0:c0 + 128],
                                        maskb[:],
                                    )
                                e = expp.tile([128, CW], ABF, tag="e")
                                nc.scalar.activation(out=e[:, c0:CW],
                                                     in_=ps_s[:, c0:CW],
                                                     func=EXP, scale=0.125)
                                if sq >= 0 and mask_select:
                                    # causal: zero exp() on the strict lower
                                    # triangle of the diagonal block
                                    nc.gpsimd.affine_select(
                                        out=e[:, c0:c0 + 128],
                                        in_=e[:, c0:c0 + 128],
                                        compare_op=mybir.AluOpType.is_ge,
                                        fill=0.0, base=0,
                                        pattern=[[1, 128]], channel_multiplier=-1,
                                    )
                                h = 2 * pair + h2
                                nc.tensor.matmul(
                                    av[h2][:, c0:CW],
                                    v_loc[i][:, h * (HD + 1):(h + 1) * (HD + 1)],
                                    e[:, c0:CW],
                                    start=(i == 0), stop=(i == n_i - 1),
                                )
                        # stage av to SBUF (frees the PSUM tile for the next
                        # pair), then normalize: o = av[0:HD] / av[HD].
                        # zrow must sit at partition 0: the approx-recip
                        # custom DVE op misbehaves on offset-64 sources.
                        for h2 in range(2):
                            h = 2 * pair + h2
                            avs = opool.tile([HD, CW], F32, tag="avs")
                            nc.scalar.copy(avs[:], av[h2][0:HD, :])
                            zrow = opool.tile([1, CW], F32, tag="zrow")
                            nc.scalar.copy(zrow[:], av[h2][HD:HD + 1, :])
                            recip = opool.tile([1, CW], F32, tag="recip")
                            if fast_recip:
                                nc.vector.reciprocal_approx_fast(
                                    out=recip[:], in_=zrow[:])
                            else:
                                nc.vector.reciprocal(recip[:], zrow[:])
                            rbc = opool.tile([HD, CW], F32, tag="rbc")
                            nc.gpsimd.partition_broadcast(rbc[:], recip[:])
                            o_j = opool.tile([HD, CW], WIRE_O, tag="o_j")
                            nc.vector.tensor_mul(o_j[:], avs[0:HD, :], rbc[:])
                            jeng.dma_start(o_shard[j][h * HD:(h + 1) * HD, :],
                                           o_j[:])

                # ====== chunk-major: qkT/v/attention/AG_o per chunk ======
                o_shard = [dram.tile([HL, CW], WIRE_O, tag=f"o_shard{j}",
                                     name=f"o_shard{j}") for j in range(NJ)]
                o_full_d = []
                for j in range(NJ):
                    jsl = slice(j * CW, (j + 1) * CW)
                    jeng = nc.sync if j == 0 else nc.scalar
                    # qkT[m rows, j] = wqk_m[:, m].T @ h[:, j]
                    for m in range(4):
                        msl = slice(m * 128, (m + 1) * 128)
                        ps = gps.tile([128, CW], F32, tag="mm")
                        for k in range(KT):
                            nc.tensor.matmul(ps[:], wqk[k][:, msl], hbf[k][:, jsl],
                                             start=(k == 0), stop=(k == KT - 1))
                        nc.scalar.copy(qkT[m][:, jsl], ps[:])

                    # v[i, head dims] = h[:, i].T @ wv_m
                    for i in range(4 * j, 4 * j + 4):
                        isl = slice(i * 128, (i + 1) * 128)
                        ps = gps.tile([128, HL], F32, tag="mm")
                        for k in range(KT):
                            nc.tensor.matmul(ps[:], hbf[k][:, isl], wv[k],
                                             start=(k == 0), stop=(k == KT - 1))
                        vv = v_loc[i][:].rearrange("p (h e) -> p h e", e=HD + 1)
                        nc.vector.tensor_copy(
                            vv[:, :, 0:HD],
                            ps[:].rearrange("p (h e) -> p h e", e=HD))

                    # attention chunk j
                    _attn_chunk(j, jeng)

                    # AG_o(j) fires as soon as chunk j's heads are written
                    ofd = dram.tile([D, CW], WIRE_O, tag=f"o_full{j}",
                                    name=f"o_full{j}")
                    o_full_d.append(ofd)
                    if no_cc == 2:
                        jeng.dma_start(ofd[0:HL, :], o_shard[j][:])
                    elif no_cc:
                        for q in range(TP):
                            jeng.dma_start(ofd[q * HL:(q + 1) * HL, :],
                                           o_shard[j][:])
                    else:
                        nc.gpsimd.collective_compute(
                            "AllGather", mybir.AluOpType.bypass,
                            replica_groups=GROUPS,
                            ins=[o_shard[j].opt()], outs=[ofd.opt()],
                        )

                # ====== proj/AG p per chunk -> residual ======
                p_full_d = []
                for j in range(NJ):
                    jeng = nc.sync if j == 0 else nc.scalar
                    ofull_all = gat.tile([128, KT, CW], WIRE_O, tag="of",
                                         name="ofull_all")
                    # AG-gated read goes on the gpsimd queue so it can't
                    # stall weight-prefetch DMAs queued on sync/scalar
                    nc.gpsimd.dma_start(
                        ofull_all[:],
                        o_full_d[j][:].rearrange("(k p) n -> p k n", p=128))
                    ofull = [ofull_all[:, k, :] for k in range(KT)]
                    p_shard = dram.tile([HL, CW], WIRE, tag=f"p_shard{j}",
                                        name=f"p_shard{j}")
                    for m in range(2):
                        msl = slice(m * 128, (m + 1) * 128)
                        ps = projps.tile([128, CW], F32, tag="pmm")
                        for k in range(KT):
                            nc.tensor.matmul(ps[:], wp[k][:, msl], ofull[k],
                                             start=(k == 0), stop=(k == KT - 1))
                        prm = prs.tile([128, CW], WIRE, tag="pr", name="pr")
                        nc.scalar.copy(prm[:], ps[:])
                        jeng.dma_start(p_shard[m * 128:(m + 1) * 128, :],
                                       prm[:])

                    pfd = dram.tile([D, CW], WIRE, tag=f"p_full{j}",
                                    name=f"p_full{j}")
                    p_full_d.append(pfd)
                    if no_cc == 2:
                        jeng.dma_start(pfd[0:HL, :], p_shard[:])
                    elif no_cc:
                        for q in range(TP):
                            jeng.dma_start(pfd[q * HL:(q + 1) * HL, :],
                                           p_shard[:])
                    else:
                        nc.gpsimd.collective_compute(
                            "AllGather", mybir.AluOpType.bypass,
                            replica_groups=GROUPS,
                            ins=[p_shard.opt()], outs=[pfd.opt()],
                        )
                for j in range(NJ):
                    jsl = slice(j * CW, (j + 1) * CW)
                    jeng = nc.sync if j == 0 else nc.scalar
                    pf_all = pf.tile([128, KT, CW], WIRE, tag="pf", name="pf_all")
                    nc.gpsimd.dma_start(
                        pf_all[:],
                        p_full_d[j][:].rearrange("(k p) n -> p k n", p=128))
                    for k in range(KT):
                        nc.vector.tensor_add(hT[k][:, jsl], hT[k][:, jsl],
                                             pf_all[:, k, :])
                        if l == n_layers - 1:
                            jeng.dma_start(outT[k * 128:(k + 1) * 128, jsl],
                                           hT[k][:, jsl])
                        else:
                            nc.vector.tensor_copy(hbf[k][:, jsl],
                                                  hT[k][:, jsl])

    nc.compile()
    return nc


def make_in_maps(inputs: dict, n_layers: int = L):
    import ml_dtypes
    BF = ml_dtypes.bfloat16
    x = np.asarray(inputs["x"], np.float32)
    Wqkv = np.asarray(inputs["Wqkv"]).astype(BF)[:n_layers]
    Aqkv = np.asarray(inputs["Aqkv"]).astype(BF)[:n_layers]
    Bqkv = np.asarray(inputs["Bqkv"]).astype(BF)[:n_layers]
    Wproj = np.asarray(inputs["Wproj"]).astype(BF)[:n_layers]
    Aproj = np.asarray(inputs["Aproj"]).astype(BF)[:n_layers]
    Bproj = np.asarray(inputs["Bproj"]).astype(BF)[:n_layers]

    in_maps = []
    for c in range(N_CORES):
        b, t = c // TP, c % TP
        cs = slice(HL * t, HL * t + HL)  # this core's head-dim columns
        wqk = np.concatenate([Wqkv[:, :, cs], Wqkv[:, :, D + HL * t:D + HL * t + HL]],
                             axis=2)
        bqk = np.concatenate([Bqkv[:, :, cs], Bqkv[:, :, D + HL * t:D + HL * t + HL]],
                             axis=2)
        in_maps.append({
            "xT": np.ascontiguousarray(x[b].T),
            "wqk": np.ascontiguousarray(wqk),
            "wv": np.ascontiguousarray(Wqkv[:, :, 2 * D + HL * t:2 * D + HL * t + HL]),
            "wp": np.ascontiguousarray(Wproj[:, :, cs]),
            "aqt": np.ascontiguousarray(Aqkv.transpose(0, 2, 1)),
            "bqk": bqk,
            "bv": np.ascontiguousarray(Bqkv[:, :, 2 * D + HL * t:2 * D + HL * t + HL]),
            "apt": np.ascontiguousarray(Aproj.transpose(0, 2, 1)),
            "bp": np.ascontiguousarray(Bproj[:, :, cs]),
        })
    return in_maps


_NC_CACHE = {}


def kernel(**inputs) -> np.ndarray:
    n_layers = L
    if n_layers not in _NC_CACHE:
        _NC_CACHE[n_layers] = build_program(n_layers)
    nc = _NC_CACHE[n_layers]
    in_maps = make_in_maps(inputs, n_layers)
    res = run_bass_kernel_spmd(nc, in_maps, core_ids=list(range(N_CORES)))
    out0 = res.results[0]["outT"].T
    out1 = res.results[TP]["outT"].T
    return np.stack([out0, out1]).astype(np.float32)


if __name__ == "__main__":
    rng = np.random.default_rng(0)
    s = 0.02
    inputs = {
        "x": rng.standard_normal((2, S, D)).astype(np.float32),
        "Wqkv": (rng.standard_normal((L, D, 3 * D)) * s).astype(np.float32),
        "Aqkv": (rng.standard_normal((L, D, 16)) * s).astype(np.float32),
        "Bqkv": (rng.standard_normal((L, 16, 3 * D)) * s).astype(np.float32),
        "Wproj": (rng.standard_normal((L, D, D)) * s).astype(np.float32),
        "Aproj": (rng.standard_normal((L, D, 16)) * s).astype(np.float32),
        "Bproj": (rng.standard_normal((L, 16, D)) * s).astype(np.float32),
    }
    out = kernel(**inputs)
    print("kernel output:", out.shape, out.dtype, float(np.abs(out).max()))



# revision 51
# speedup vs baseline: 1.0412x; 1.0343x over previous
"""Trainium2 Bass kernel for an 8-layer LoRA attention model.

Model (per layer): Wq_m = Wqkv + Aqkv@Bqkv; qkv = h @ Wq_m; causal MHA
(16 heads, hd=64); Wp_m = Wproj + Aproj@Bproj; h = h + attn_out @ Wp_m.
x: [2, 1024, 1024] fp32.

Distribution (8 cores): DP2 x TP4. Cores 0-3 handle batch 0, cores 4-7
batch 1. Within a group each core owns 4 heads: a 256-column shard of
Wq/Wk/Wv (Megatron split) and a 256-OUTPUT-column shard of Wproj. Per
layer the group does two AllGathers (attention outputs, then proj
outputs); there is no reduce, so results are bit-identical across the
group and numerically equal to the unsharded computation.

Layout: the residual stream is kept transposed on-chip: hT [1024 d, 1024
tokens]. The qk GEMM produces qT/kT directly ([dims, tokens]); v is
produced untransposed ([tokens, dims]) so it can serve as the stationary
operand of the attention AV matmul. Attention uses the scoresT [s2, s1]
orientation; softmax row-sums come from a ones-column appended to v
(AV output row 64 = sum of exp), so no partition-dim reductions or
transposes are ever needed.

Activations run in float32r (fp32 storage, ~12-bit mantissa operand
rounding, full PE rate); weights are shipped and used as bf16 stationary
operands (same PE rate, half the wire/DMA/LDWEIGHTS cost). All matmuls
accumulate in fp32 PSUM.
"""

import numpy as np

import concourse.bass as bass
import concourse.mybir as mybir
import concourse.tile as tile
from concourse import bacc
from concourse.bass_utils import run_bass_kernel_spmd

F32 = mybir.dt.float32
BF16 = mybir.dt.bfloat16
F32R = mybir.dt.float32r
F8 = mybir.dt.float8e4
EXP = mybir.ActivationFunctionType.Exp

L = 8          # layers
D = 1024       # model dim
S = 1024       # sequence (tokens per batch == tokens per core)
HD = 64        # head dim
HPC = 4        # heads per core
HL = HPC * HD  # local head dims (256)
TP = 4         # tensor-parallel group size
N_CORES = 8
GROUPS = [[0, 1, 2, 3], [4, 5, 6, 7]]
NEG = -1.0e30

KT = D // 128   # k tiles of the model dim (8)
NJ = 2          # token chunks (512 each)
CW = S // NJ    # chunk width (512)


def build_program(n_layers: int = L, reps: int = 1, no_cc: bool = False,
                  num_devices: int = N_CORES, wire_bf16: bool = True,
                  attn_bf16: bool = True, gps_bufs: int = 2, avps_bufs: int = 2,
                  fast_recip: bool = True, mask_select: bool = True,
                  o_fp8: bool = False):
    nc = bacc.Bacc("TRN2", target_bir_lowering=False, debug=False,
                   num_devices=num_devices)
    WIRE = BF16 if wire_bf16 else F32R
    WIRE_O = F8 if o_fp8 else WIRE  # attention-output wire (AG + proj moving)
    ABF = BF16 if attn_bf16 else F32R

    # ---- per-core external inputs (host pre-sharded; weights/x bf16) ----
    xT = nc.dram_tensor("xT", [D, S], BF16, kind="ExternalInput")
    wqk_d = nc.dram_tensor("wqk", [n_layers, D, 2 * HL], BF16, kind="ExternalInput")
    wv_d = nc.dram_tensor("wv", [n_layers, D, HL], BF16, kind="ExternalInput")
    wp_d = nc.dram_tensor("wp", [n_layers, D, HL], BF16, kind="ExternalInput")
    aqt_d = nc.dram_tensor("aqt", [n_layers, 16, D], BF16, kind="ExternalInput")
    bqk_d = nc.dram_tensor("bqk", [n_layers, 16, 2 * HL], BF16, kind="ExternalInput")
    bv_d = nc.dram_tensor("bv", [n_layers, 16, HL], BF16, kind="ExternalInput")
    apt_d = nc.dram_tensor("apt", [n_layers, 16, D], BF16, kind="ExternalInput")
    bp_d = nc.dram_tensor("bp", [n_layers, 16, HL], BF16, kind="ExternalInput")
    outT = nc.dram_tensor("outT", [D, S], F32R, kind="ExternalOutput")

    with tile.TileContext(nc) as tc:
        from contextlib import ExitStack
        with ExitStack() as _st:
            _p = lambda *a, **kw: _st.enter_context(tc.tile_pool(*a, **kw))
            per = _p(name="per", bufs=1)          # persistent
            wpool = _p(name="wpool", bufs=2)      # qk weights (prefetch)
            wpool1 = _p(name="wpool1", bufs=2)    # v/p weights (prefetch)
            lora = _p(name="lora", bufs=2)
            lora2 = _p(name="lora2", bufs=2)
            expp = _p(name="expp", bufs=3)
            opool = _p(name="opool", bufs=2)      # o chunks + norm temps
            gat = _p(name="gat", bufs=1)          # gathered o (per chunk)
            pf = _p(name="pf", bufs=2 if wire_bf16 else 1)  # gathered proj
            prs = _p(name="prs", bufs=2)          # proj shard staging
            qkv2 = _p(name="qkv2", bufs=2)
            gps = _p(name="gps", bufs=gps_bufs, space="PSUM")
            projps = _p(name="projps", bufs=2, space="PSUM")
            sps = _p(name="sps", bufs=2, space="PSUM")
            avps = _p(name="avps", bufs=avps_bufs, space="PSUM")
            dram = _p(name="dram", bufs=2, space="DRAM")
            # ---- persistent state ----
            hT = [per.tile([128, S], F32R, tag=f"hT{k}", name=f"hT{k}") for k in range(KT)]
            # bf16 shadow of the residual: the matmul-operand copy
            hbf = [per.tile([128, S], BF16, tag=f"hbf{k}", name=f"hbf{k}")
                   for k in range(KT)]

            ones_sb = per.tile([128, HPC, 1], ABF, tag="ones", name="ones_sb")
            nc.gpsimd.memset(ones_sb[:], 1.0)

            # causal bias: [128,128] lower-strict triangle = NEG, else 0
            maskb = per.tile([128, 128], F32, tag="maskb", name="maskb")
            nc.gpsimd.memset(maskb[:], 0.0)
            nc.gpsimd.affine_select(
                out=maskb[:], in_=maskb[:],
                compare_op=mybir.AluOpType.is_ge,
                fill=NEG, base=0,
                pattern=[[1, 128]], channel_multiplier=-1,
            )

            for rep in range(reps):
              for jk in range(NJ * KT):
                  j, k = divmod(jk, KT)
                  jsl = slice(j * CW, (j + 1) * CW)
                  nc.sync.dma_start(hbf[k][:, jsl],
                                    xT[k * 128:(k + 1) * 128, jsl])
                  nc.vector.tensor_copy(hT[k][:, jsl], hbf[k][:, jsl])
              for l in range(n_layers):
                # qkT [512 rows, S]: rows 0..255 = qT (4 heads x 64),
                # 256..511 = kT.  4 tiles of [128, S].  Double-buffered so
                # next layer's GEMMs overlap this layer's attention reads.
                qkT = [qkv2.tile([128, S], ABF, tag=f"qkT{m}", name=f"qkT{m}")
                       for m in range(4)]
                # v_loc per s2-tile: [128, 4*65]; per head 64 v-dims + ones col
                v_loc = [qkv2.tile([128, HPC * (HD + 1)], ABF, tag=f"v{i}",
                                   name=f"v{i}") for i in range(S // 128)]
                for i in range(S // 128):
                    vv = v_loc[i][:].rearrange("p (h e) -> p h e", e=HD + 1)
                    nc.vector.tensor_copy(vv[:, :, HD:HD + 1], ones_sb[:])
                # ================= weights + LoRA merge =================
                wqk_all = wpool.tile([128, KT, 2 * HL], BF16, tag="wqk",
                                     name="wqk_all")
                wv_all = wpool1.tile([128, KT, HL], BF16, tag="wv", name="wv_all")
                wp_all = wpool1.tile([128, KT, HL], BF16, tag="wp", name="wp_all")
                nc.sync.dma_start(
                    wqk_all[:], wqk_d[l].rearrange("(k p) n -> p k n", p=128))
                nc.sync.dma_start(
                    wv_all[:], wv_d[l].rearrange("(k p) n -> p k n", p=128))
                nc.sync.dma_start(
                    wp_all[:], wp_d[l].rearrange("(k p) n -> p k n", p=128))
                wqk = [wqk_all[:, k, :] for k in range(KT)]
                wv = [wv_all[:, k, :] for k in range(KT)]
                wp = [wp_all[:, k, :] for k in range(KT)]
                aqt = lora.tile([16, D], BF16, tag="aqt", name="aqt")
                apt = lora2.tile([16, D], BF16, tag="apt", name="apt")
                nc.sync.dma_start(aqt[:], aqt_d[l])
                nc.sync.dma_start(apt[:], apt_d[l])
                bqk = lora.tile([16, 2 * HL], BF16, tag="bqk", name="bqk")
                bv = lora.tile([16, HL], BF16, tag="bv", name="bv")
                bp = lora2.tile([16, HL], BF16, tag="bp", name="bp")
                nc.sync.dma_start(bqk[:], bqk_d[l])
                nc.sync.dma_start(bv[:], bv_d[l])
                nc.sync.dma_start(bp[:], bp_d[l])

                # LoRA merge (runs a layer ahead, off the critical path):
                # W[k] += A^T[:, k].T @ B for each 128-row tile of each W
                for k in range(KT):
                    ksl = slice(k * 128, (k + 1) * 128)
                    mq = gps.tile([128, 2 * HL], F32, tag="mm", name="mq")
                    nc.tensor.matmul(mq[:], aqt[:, ksl], bqk[:],
                                     start=True, stop=True)
                    nc.vector.tensor_add(wqk[k], wqk[k], mq[:])
                    mv = gps.tile([128, HL], F32, tag="mm", name="mv")
                    nc.tensor.matmul(mv[:], aqt[:, ksl], bv[:],
                                     start=True, stop=True)
                    nc.vector.tensor_add(wv[k], wv[k], mv[:])
                    mp = gps.tile([128, HL], F32, tag="mm", name="mp")
                    nc.tensor.matmul(mp[:], apt[:, ksl], bp[:],
                                     start=True, stop=True)
                    nc.vector.tensor_add(wp[k], wp[k], mp[:])

                def _attn_chunk(j, jeng):
                    n_i = 4 * j + 4  # causal: s2-tiles 0 .. 4j+3
                    for pair in range(2):
                        av = [avps.tile([HD + 1, CW], F32, tag="av", name="av")
                              for _ in range(2)]
                        for i in range(n_i):
                            sq = i - 4 * j  # >=0 on the diagonal band
                            c0 = max(sq, 0) * 128
                            for h2 in range(2):
                                hsl = slice(64 * h2, 64 * h2 + 64)
                                ps_s = sps.tile([128, CW], F32, tag="sc")
                                nc.tensor.matmul(
                                    ps_s[:, c0:CW],
                                    qkT[2 + pair][hsl, i * 128:(i + 1) * 128],
                                    qkT[pair][hsl, j * CW + c0:(j + 1) * CW],
                                    start=True, stop=True,
                                    tile_position=(64 * h2, 0),
                                )
                                if sq >= 0 and not mask_select:
                                    nc.vector.tensor_add(
                                        ps_s[:, c0:c0 + 128],
                                        ps_s[:, c0:c0 + 128],
                                        maskb[:],
                                    )
                                e = expp.tile([128, CW], ABF, tag="e")
                                nc.scalar.activation(out=e[:, c0:CW],
                                                     in_=ps_s[:, c0:CW],
                                                     func=EXP, scale=0.125)
                                if sq >= 0 and mask_select:
                                    # causal: zero exp() on the strict lower
                                    # triangle of the diagonal block
                                    nc.gpsimd.affine_select(
                                        out=e[:, c0:c0 + 128],
                                        in_=e[:, c0:c0 + 128],
                                        compare_op=mybir.AluOpType.is_ge,
                                        fill=0.0, base=0,
                                        pattern=[[1, 128]], channel_multiplier=-1,
                                    )
                                h = 2 * pair + h2
                                nc.tensor.matmul(
                                    av[h2][:, c0:CW],
                                    v_loc[i][:, h * (HD + 1):(h + 1) * (HD + 1)],
                                    e[:, c0:CW],
                                    start=(i == 0), stop=(i == n_i - 1),
                                )
                        # stage av to SBUF (frees the PSUM tile for the next
                        # pair), then normalize: o = av[0:HD] / av[HD].
                        # zrow must sit at partition 0: the approx-recip
                        # custom DVE op misbehaves on offset-64 sources.
                        for h2 in range(2):
                            h = 2 * pair + h2
                            avs = opool.tile([HD, CW], F32, tag="avs")
                            nc.scalar.copy(avs[:], av[h2][0:HD, :])
                            zrow = opool.tile([1, CW], F32, tag="zrow")
                            nc.scalar.copy(zrow[:], av[h2][HD:HD + 1, :])
                            recip = opool.tile([1, CW], F32, tag="recip")
                            if fast_recip:
                                nc.vector.reciprocal_approx_fast(
                                    out=recip[:], in_=zrow[:])
                            else:
                                nc.vector.reciprocal(recip[:], zrow[:])
                            rbc = opool.tile([HD, CW], F32, tag="rbc")
                            nc.gpsimd.partition_broadcast(rbc[:], recip[:])
                            o_j = opool.tile([HD, CW], WIRE_O, tag="o_j")
                            nc.vector.tensor_mul(o_j[:], avs[0:HD, :], rbc[:])
                            jeng.dma_start(o_shard[j][h * HD:(h + 1) * HD, :],
                                           o_j[:])

                # ====== chunk-major: qkT/v/attention/AG_o per chunk ======
                o_shard = [dram.tile([HL, CW], WIRE_O, tag=f"o_shard{j}",
                                     name=f"o_shard{j}") for j in range(NJ)]
                o_full_d = []
                for j in range(NJ):
                    jsl = slice(j * CW, (j + 1) * CW)
                    jeng = nc.sync if j == 0 else nc.scalar
                    # qkT[m rows, j] = wqk_m[:, m].T @ h[:, j]
                    for m in range(4):
                        msl = slice(m * 128, (m + 1) * 128)
                        ps = gps.tile([128, CW], F32, tag="mm")
                        for k in range(KT):
                            nc.tensor.matmul(ps[:], wqk[k][:, msl], hbf[k][:, jsl],
                                             start=(k == 0), stop=(k == KT - 1))
                        nc.scalar.copy(qkT[m][:, jsl], ps[:])

                    # v[i, head dims] = h[:, i].T @ wv_m
                    for i in range(4 * j, 4 * j + 4):
                        isl = slice(i * 128, (i + 1) * 128)
                        ps = gps.tile([128, HL], F32, tag="mm")
                        for k in range(KT):
                            nc.tensor.matmul(ps[:], hbf[k][:, isl], wv[k],
                                             start=(k == 0), stop=(k == KT - 1))
                        vv = v_loc[i][:].rearrange("p (h e) -> p h e", e=HD + 1)
                        nc.vector.tensor_copy(
                            vv[:, :, 0:HD],
                            ps[:].rearrange("p (h e) -> p h e", e=HD))

                    # attention chunk j
                    _attn_chunk(j, jeng)

                    # AG_o(j) fires as soon as chunk j's heads are written
                    ofd = dram.tile([D, CW], WIRE_O, tag=f"o_full{j}",
                                    name=f"o_full{j}")
                    o_full_d.append(ofd)
                    if no_cc == 2:
                        jeng.dma_start(ofd[0:HL, :], o_shard[j][:])
                    elif no_cc:
                        for q in range(TP):
                            jeng.dma_start(ofd[q * HL:(q + 1) * HL, :],
                                           o_shard[j][:])
                    else:
                        nc.gpsimd.collective_compute(
                            "AllGather", mybir.AluOpType.bypass,
                            replica_groups=GROUPS,
                            ins=[o_shard[j].opt()], outs=[ofd.opt()],
                        )

                # ====== proj/AG p per chunk -> residual ======
                p_full_d = []
                for j in range(NJ):
                    jeng = nc.sync if j == 0 else nc.scalar
                    ofull_all = gat.tile([128, KT, CW], WIRE_O, tag="of",
                                         name="ofull_all")
                    # AG-gated read goes on the gpsimd queue so it can't
                    # stall weight-prefetch DMAs queued on sync/scalar
                    nc.gpsimd.dma_start(
                        ofull_all[:],
                        o_full_d[j][:].rearrange("(k p) n -> p k n", p=128))
                    ofull = [ofull_all[:, k, :] for k in range(KT)]
                    p_shard = dram.tile([HL, CW], WIRE, tag=f"p_shard{j}",
                                        name=f"p_shard{j}")
                    for m in range(2):
                        msl = slice(m * 128, (m + 1) * 128)
                        ps = projps.tile([128, CW], F32, tag="pmm")
                        for k in range(KT):
                            nc.tensor.matmul(ps[:], wp[k][:, msl], ofull[k],
                                             start=(k == 0), stop=(k == KT - 1))
                        prm = prs.tile([128, CW], WIRE, tag="pr", name="pr")
                        nc.scalar.copy(prm[:], ps[:])
                        jeng.dma_start(p_shard[m * 128:(m + 1) * 128, :],
                                       prm[:])

                    pfd = dram.tile([D, CW], WIRE, tag=f"p_full{j}",
                                    name=f"p_full{j}")
                    p_full_d.append(pfd)
                    if no_cc == 2:
                        jeng.dma_start(pfd[0:HL, :], p_shard[:])
                    elif no_cc:
                        for q in range(TP):
                            jeng.dma_start(pfd[q * HL:(q + 1) * HL, :],
                                           p_shard[:])
                    else:
                        nc.gpsimd.collective_compute(
                            "AllGather", mybir.AluOpType.bypass,
                            replica_groups=GROUPS,
                            ins=[p_shard.opt()], outs=[pfd.opt()],
                        )
                for j in range(NJ):
                    jsl = slice(j * CW, (j + 1) * CW)
                    jeng = nc.sync if j == 0 else nc.scalar
                    pf_all = pf.tile([128, KT, CW], WIRE, tag="pf", name="pf_all")
                    nc.gpsimd.dma_start(
                        pf_all[:],
                        p_full_d[j][:].rearrange("(k p) n -> p k n", p=128))
                    for k in range(KT):
                        nc.vector.tensor_add(hT[k][:, jsl], hT[k][:, jsl],
                                             pf_all[:, k, :])
                        if l == n_layers - 1:
                            jeng.dma_start(outT[k * 128:(k + 1) * 128, jsl],
                                           hT[k][:, jsl])
                        else:
                            nc.vector.tensor_copy(hbf[k][:, jsl],
                                                  hT[k][:, jsl])

    nc.compile()
    return nc


def make_in_maps(inputs: dict, n_layers: int = L):
    import ml_dtypes
    BF = ml_dtypes.bfloat16
    x = np.asarray(inputs["x"], np.float32)
    Wqkv = np.asarray(inputs["Wqkv"]).astype(BF)[:n_layers]
    Aqkv = np.asarray(inputs["Aqkv"]).astype(BF)[:n_layers]
    Bqkv = np.asarray(inputs["Bqkv"]).astype(BF)[:n_layers]
    Wproj = np.asarray(inputs["Wproj"]).astype(BF)[:n_layers]
    Aproj = np.asarray(inputs["Aproj"]).astype(BF)[:n_layers]
    Bproj = np.asarray(inputs["Bproj"]).astype(BF)[:n_layers]

    in_maps = []
    for c in range(N_CORES):
        b, t = c // TP, c % TP
        cs = slice(HL * t, HL * t + HL)  # this core's head-dim columns
        wqk = np.concatenate([Wqkv[:, :, cs], Wqkv[:, :, D + HL * t:D + HL * t + HL]],
                             axis=2)
        bqk = np.concatenate([Bqkv[:, :, cs], Bqkv[:, :, D + HL * t:D + HL * t + HL]],
                             axis=2)
        in_maps.append({
            "xT": np.ascontiguousarray(x[b].T).astype(BF),
            "wqk": np.ascontiguousarray(wqk),
            "wv": np.ascontiguousarray(Wqkv[:, :, 2 * D + HL * t:2 * D + HL * t + HL]),
            "wp": np.ascontiguousarray(Wproj[:, :, cs]),
            "aqt": np.ascontiguousarray(Aqkv.transpose(0, 2, 1)),
            "bqk": bqk,
            "bv": np.ascontiguousarray(Bqkv[:, :, 2 * D + HL * t:2 * D + HL * t + HL]),
            "apt": np.ascontiguousarray(Aproj.transpose(0, 2, 1)),
            "bp": np.ascontiguousarray(Bproj[:, :, cs]),
        })
    return in_maps


_NC_CACHE = {}


def kernel(**inputs) -> np.ndarray:
    n_layers = L
    if n_layers not in _NC_CACHE:
        _NC_CACHE[n_layers] = build_program(n_layers)
    nc = _NC_CACHE[n_layers]
    in_maps = make_in_maps(inputs, n_layers)
    res = run_bass_kernel_spmd(nc, in_maps, core_ids=list(range(N_CORES)))
    out0 = res.results[0]["outT"].T
    out1 = res.results[TP]["outT"].T
    return np.stack([out0, out1]).astype(np.float32)


if __name__ == "__main__":
    rng = np.random.default_rng(0)
    s = 0.02
    inputs = {
        "x": rng.standard_normal((2, S, D)).astype(np.float32),
        "Wqkv": (rng.standard_normal((L, D, 3 * D)) * s).astype(np.float32),
        "Aqkv": (rng.standard_normal((L, D, 16)) * s).astype(np.float32),
        "Bqkv": (rng.standard_normal((L, 16, 3 * D)) * s).astype(np.float32),
        "Wproj": (rng.standard_normal((L, D, D)) * s).astype(np.float32),
        "Aproj": (rng.standard_normal((L, D, 16)) * s).astype(np.float32),
        "Bproj": (rng.standard_normal((L, 16, D)) * s).astype(np.float32),
    }
    out = kernel(**inputs)
    print("kernel output:", out.shape, out.dtype, float(np.abs(out).max()))

